# revision 1
# baseline (speedup 1.0000x reference)
"""Trainium2 Bass kernel for nn_CausalAttentionKVCache (B=2, T=2048, D=1024, 16 heads).

Sharding: 8 cores = 2 batch-halves x 4 head-groups (4 heads each).
Two compiled SPMD programs (one per batch-half, phase constants differ mod 3),
dispatched concurrently on jax devices [0:4] and [4:8].

The module's reshape y.view(3,B,T,hs,nh) scrambles tokens: flat row
v = (c*B*T + b*T + t)//3 of y=[x@W+b] in column block j=(c*B*T+b*T+t)%3 holds
token t of tensor c (q/k/v). With a host-side column permutation of W
(W2[:, j*1024+h*64+d] = W[:, j*1024+d*16+h]) each head's 64 features are
contiguous, and each token-residue class (t mod 3) is a contiguous row-run.

Per core: YT_qk = WQK^T @ xT (features on partitions) feeds Q^T (strided
descramble on PSUM eviction) and K^T (contiguous, v-indexed); V is projected
per-residue with a shifted v-window so its rows align with the k-chunk grid,
with a ones-column per head for the softmax denominator. Attention runs in
S^T = K^T.T@Q^T layout (k on partitions): exp on ScalarE (scale=1/8 fused, no
max-subtraction needed: scores ~ N(0,1)), causal staircase zeroed by gpsimd
affine_select, PV with V stationary accumulating ctx^T, PE-transpose + per-
partition reciprocal for the final division. Matmuls use float32r
(~1.5e-4 rel err, 4x fp32 throughput).
"""
import sys
import os

sys.path.insert(0, "/opt/trn_rl_repo")

import numpy as np

import concourse.bass as bass
import concourse.bacc as bacc
import concourse.mybir as mybir
import concourse.tile as tile
from concourse.masks import make_identity

B, T, D, NH, HS = 2, 2048, 1024, 16, 64
NV = 684          # v-rows per (c, batch-half) slice
NVV = 772         # XTV slice width (guard col + 768 window + pad)
GUARD = 1
NCHUNK = 6        # k/v chunks of 128 rows
QW = 512          # q window
F32R = mybir.dt.float32r
F32 = mybir.dt.float32

_CACHE = {}


def _phase(B2):
    """Compile-time residue/offset constants for batch-half B2."""
    cst = {}
    for c in range(3):
        u0 = c * B * T + B2 * T
        vstart = u0 // 3
        rc_of_jj, r0_of_jj = {}, {}
        for rc in range(3):
            jj = (u0 + rc) % 3
            rc_of_jj[jj] = rc
            r0_of_jj[jj] = (u0 + rc - jj) // 3 - vstart
        cst[c] = dict(u0=u0, vstart=vstart, rc=rc_of_jj, r0=r0_of_jj)
    # rc-indexed views
    jk = {cst[1]["rc"][j]: j for j in range(3)}
    r0k = {cst[1]["rc"][j]: cst[1]["r0"][j] for j in range(3)}
    jv = {cst[2]["rc"][j]: j for j in range(3)}
    r0v = {cst[2]["rc"][j]: cst[2]["r0"][j] for j in range(3)}
    return cst, jk, r0k, jv, r0v


def _chunks(B2, q0):
    """Valid k-chunks (m, rc) for q-window [q0, q0+QW), with extents."""
    _, jk, r0k, _, _ = _phase(B2)
    out = []
    for m in range(NCHUNK):
        for rc in range(3):
            t_min = rc + 3 * (128 * m - r0k[rc])
            if t_min >= q0 + QW:
                continue
            a = max(0, t_min - q0) & ~1
            out.append((m, rc, t_min, a))
    return out


def _build_program(B2, repeat=1):
    cst, jk, r0k, jv, r0v = _phase(B2)
    nc = bacc.Bacc("TRN2", target_bir_lowering=False, debug=False, num_devices=4)

    xtq_d = nc.dram_tensor("XTQ", [D, 768], F32R, kind="ExternalInput")
    xtk_d = nc.dram_tensor("XTK", [D, 768], F32R, kind="ExternalInput")
    xtv_d = nc.dram_tensor("XTV", [D, NVV], F32R, kind="ExternalInput")
    wqk_d = nc.dram_tensor("WQK", [D, 768], F32R, kind="ExternalInput")
    wv_d = nc.dram_tensor("WV", [D, 780], F32R, kind="ExternalInput")
    bqk_d = nc.dram_tensor("BQK", [128, 6], F32, kind="ExternalInput")
    bv_d = nc.dram_tensor("BV", [1, 780], F32R, kind="ExternalInput")
    ones_d = nc.dram_tensor("ONES", [1, 128], F32R, kind="ExternalInput")
    zeros_d = nc.dram_tensor("ZEROS", [128, 260], F32R, kind="ExternalInput")
    out_d = nc.dram_tensor("OUT", [T, 256], F32, kind="ExternalOutput")

    with tile.TileContext(nc) as tc:
        with (
            tc.tile_pool(name="const", bufs=1) as constp,
            tc.tile_pool(name="wpool", bufs=1) as wpool,
            tc.tile_pool(name="xpool", bufs=2) as xpool,
            tc.tile_pool(name="qkv", bufs=1) as qkvp,
            tc.tile_pool(name="ppool", bufs=6) as ppool,
            tc.tile_pool(name="cxpool", bufs=2) as cxpool,
            tc.tile_pool(name="opool", bufs=2) as opool,
            tc.tile_pool(name="rpool", bufs=2) as rpool,
        ):
            from contextlib import ExitStack
            identity = constp.tile([128, 128], F32)
            make_identity(nc, identity[:])
            ones = constp.tile([1, 128], F32R)
            nc.sync.dma_start(ones[:], ones_d[:, :])
            zeros = constp.tile([128, 260], F32R)
            nc.sync.dma_start(zeros[:], zeros_d[:, :])

            wqk = wpool.tile([128, 8, 768], F32R)
            wv = wpool.tile([128, 8, 780], F32R)
            bqk = wpool.tile([128, 6], F32)
            bv = wpool.tile([1, 780], F32R)
            nc.sync.dma_start(bqk[:], bqk_d[:, :])
            nc.sync.dma_start(bv[:], bv_d[:, :])
            for ic in range(8):
                nc.sync.dma_start(
                    wqk[:, ic, :],
                    wqk_d.rearrange("(c p) f -> p c f", p=128)[:, ic, :])

            for _rep in range(repeat):
                proj_ctx = ExitStack()
                psqk = proj_ctx.enter_context(
                    tc.tile_pool(name="psqk", bufs=3, space="PSUM"))
                qt = qkvp.tile([128, 2, T], F32R, tag="qt")
                kt = qkvp.tile([128, 2, 3, 768], F32R, tag="kt")
                yvs = qkvp.tile([128, NCHUNK, 3, 260], F32R, tag="yvs")

                xts = {}
                for si, (src_d, name) in enumerate(
                        [(xtq_d, "xq"), (xtk_d, "xk")]):
                    xt = xpool.tile([128, 8, NVV], F32R, tag="xt", name=name)
                    for ic in range(8):
                        nc.sync.dma_start(
                            xt[:, ic, :768],
                            src_d.rearrange("(c p) v -> p c v", p=128)[:, ic, :])
                    xts[si] = xt

                # ---- projection group emitters ----
                def emit_qk_group(si, fc, pool=None, tag="psqk"):
                    xt = xts[si]
                    ps = (pool or psqk).tile([128, 1024], F32, tag=tag,
                                             name="psqk")
                    for v0, v1 in ((0, 512), (512, 768)):
                        for ic in range(8):
                            nc.tensor.matmul(
                                ps[:, v0:v1],
                                wqk[:, ic, fc * 128:(fc + 1) * 128],
                                xt[:, ic, v0:v1],
                                start=(ic == 0),
                                stop=(ic == 7),
                            )
                    jj, hp = fc // 2, fc % 2
                    if si == 0:   # Q: strided descramble eviction + bias
                        rc, r0 = cst[0]["rc"][jj], cst[0]["r0"][jj]
                        nrc = 683 if rc < 2 else 682
                        vlo, vhi = r0, min(NV, r0 + nrc)
                        n = vhi - vlo
                        nc.vector.tensor_scalar_add(
                            qt[:, hp, rc: min(rc + 3 * n, T): 3],
                            ps[:, vlo:vhi],
                            bqk[:, fc: fc + 1],
                        )
                    else:         # K: contiguous, v-indexed
                        nc.vector.tensor_scalar_add(
                            kt[:, hp, jj, 0:NV],
                            ps[:, 0:NV],
                            bqk[:, fc: fc + 1],
                        )
                        if jj == 2:
                            for jz in range(3):
                                nc.vector.tensor_copy(
                                    kt[:, hp, jz, NV:768],
                                    zeros[:, 0:768 - NV])

                def emit_v_group(rc, m, pool=None, tag="psqk"):
                    jjv = jv[rc]
                    delta = r0v[rc] - r0k[rc]
                    r0 = r0k[rc]
                    nrc = 683 if rc < 2 else 682
                    lim = r0 + nrc
                    mlo, plo = divmod(lim, 128)
                    if m > mlo or (m == mlo and plo == 0):
                        nc.vector.tensor_copy(yvs[:, m, rc, :], zeros[:])
                        return
                    ps = (pool or psqk).tile([128, 1024], F32, tag=tag,
                                             name="psv")
                    x0 = GUARD + 128 * m + delta
                    for ic in range(8):
                        nc.tensor.matmul(
                            ps[:, 0:260],
                            xtv[:, ic, x0: x0 + 128],
                            wv[:, ic, jjv * 260:(jjv + 1) * 260],
                            start=(ic == 0),
                            stop=False,
                        )
                    nc.tensor.matmul(
                        ps[:, 0:260],
                        ones[0:1, 0:128],
                        bv[0:1, jjv * 260:(jjv + 1) * 260],
                        start=False,
                        stop=True,
                    )
                    if m == mlo:
                        nc.vector.tensor_copy(yvs[:, m, rc, :], zeros[:])
                        nc.vector.tensor_copy(
                            yvs[0:plo, m, rc, :], ps[0:plo, 0:260])
                    else:
                        nc.vector.tensor_copy(yvs[:, m, rc, :], ps[:, 0:260])
                        if m == 0 and r0 > 0:
                            nc.vector.tensor_copy(
                                yvs[0:r0, 0, rc, :], zeros[0:r0, :])

                # ---- attention emitters ----
                def emit_s_exp(hp, q0, chunk):
                    m, rc, t_min, a = chunk
                    a = min(a, QW - 256)   # keep matmul N >= 256 (f32r rate)
                    jjk, r0 = jk[rc], r0k[rc]
                    s_ps = pss.tile([128, 2 * QW], F32, tag="s", name="s_ps")
                    for hr in range(2):
                        pr = slice(hr * 64, hr * 64 + 64)
                        nc.tensor.matmul(
                            s_ps[:, hr * QW + a: (hr + 1) * QW],
                            kt[pr, hp, jjk, 128 * m: 128 * (m + 1)],
                            qt[pr, hp, q0 + a: q0 + QW],
                            start=True,
                            stop=True,
                            tile_position=(hr * 64, 0),
                        )
                    p_sb = ppool.tile([128, 2, QW], F32R, tag="p", name="p_sb")
                    s3 = s_ps[:].rearrange("p (h w) -> p h w", h=2)
                    nc.scalar.activation(
                        p_sb[:, :, a:QW],
                        s3[:, :, a:QW],
                        mybir.ActivationFunctionType.Exp,
                        scale=float(HS) ** -0.5,
                    )
                    ws, we = a, min(QW, t_min + 382 - q0)
                    if ws < we:
                        nc.gpsimd.affine_select(
                            out=p_sb[:, :, ws:we],
                            in_=p_sb[:, :, ws:we],
                            pattern=[[0, 2], [1, we - ws]],
                            compare_op=mybir.AluOpType.is_ge,
                            fill=0.0,
                            base=q0 + ws - rc - 384 * m + 3 * r0,
                            channel_multiplier=-3,
                        )
                    return p_sb

                def emit_pv(hp, ctx, nchunks, ci, chunk, p_sb):
                    m, rc, t_min, a = chunk
                    a = min(a, QW - 256)
                    for hr in range(2):
                        h_loc = 2 * hp + hr
                        nc.tensor.matmul(
                            ctx[hr][:, a:QW],
                            yvs[:, m, rc, h_loc * 65:(h_loc + 1) * 65],
                            p_sb[:, hr, a:QW],
                            start=(ci == 0),
                            stop=(ci == nchunks - 1),
                        )

                def make_epilogue(hp, q0, ctx):
                    def epi():
                        cx = cxpool.tile([65, 2, QW], F32, tag="cx", name="cx")
                        for hr in range(2):
                            nc.vector.tensor_copy(cx[:, hr, :], ctx[hr][:])
                        for hr in range(2):
                            o_sb = opool.tile([128, 4, 64], F32, tag="o",
                                              name="o_sb")
                            for qb in range(QW // 128):
                                tr = pss.tile([128, 65], F32, tag="s",
                                              name="tr")
                                nc.tensor.transpose(
                                    tr[:],
                                    cx[0:65, hr, qb * 128:(qb + 1) * 128],
                                    identity[0:65, 0:65],
                                )
                                rec = rpool.tile([128, 1], F32, tag="rec",
                                                 name="rec")
                                nc.vector.reciprocal(rec[:], tr[:, 64:65])
                                nc.vector.tensor_scalar_mul(
                                    o_sb[:, qb, :], tr[:, 0:64], rec[:]
                                )
                            nc.sync.dma_start(
                                out_d[q0: q0 + QW, (2 * hp + hr) * 64:
                                      (2 * hp + hr + 1) * 64].rearrange(
                                    "(qb p) d -> p qb d", p=128
                                ),
                                o_sb[:],
                            )
                    return epi

                # ---- emission schedule ----
                # lead-in: all Q projections (frees the xq slot for xv),
                # K projections for hp=0, V chunks m=0,1
                for fc in (0, 2, 4, 1, 3, 5):
                    emit_qk_group(0, fc)
                xtv = xpool.tile([128, 8, NVV], F32R, tag="xt", name="xv")
                for ic in range(8):
                    nc.sync.dma_start(
                        wv[:, ic, :],
                        wv_d.rearrange("(c p) f -> p c f", p=128)[:, ic, :])
                    nc.sync.dma_start(
                        xtv[:, ic, :],
                        xtv_d.rearrange("(c p) v -> p c v", p=128)[:, ic, :])
                for fc in (0, 2, 4):
                    emit_qk_group(1, fc)
                for m in (0, 1):
                    for rc in range(3):
                        emit_v_group(rc, m)
                proj_ctx.close()
                attn_ctx = ExitStack()
                pss = attn_ctx.enter_context(
                    tc.tile_pool(name="pss", bufs=3, space="PSUM"))
                psctx = attn_ctx.enter_context(
                    tc.tile_pool(name="psctx", bufs=2, space="PSUM"))

                # fillers sprinkled into attention windows of hp=0
                # (K for hp=1 and remaining V chunks; psum from the s pool)
                def fqk(fc):
                    return lambda: emit_qk_group(1, fc, pool=pss, tag="s")

                def fv(rc, m):
                    return lambda: emit_v_group(rc, m, pool=pss, tag="s")

                fillers = {
                    0: [fv(rc, 2) for rc in range(3)] + [fqk(1)],
                    1: [fv(rc, 3) for rc in range(3)] + [fqk(3), fqk(5)],
                    2: [fv(rc, m) for m in (4, 5) for rc in range(3)],
                }

                DEPTH = 3
                deferred_epi = None
                for hp in range(2):
                    for qi, q0 in enumerate(range(0, T, QW)):
                        chunks = _chunks(B2, q0)
                        fill = list(fillers.get(qi, [])) if hp == 0 else []
                        ctx = [
                            psctx.tile([65, QW], F32, tag="ctx",
                                       name=f"ctx{hr}")
                            for hr in range(2)
                        ]
                        pend = []
                        for ci in range(len(chunks)):
                            pend.append((ci, chunks[ci],
                                         emit_s_exp(hp, q0, chunks[ci])))
                            if ci == 4 and deferred_epi is not None:
                                deferred_epi()
                                deferred_epi = None
                            if fill and ci % 2 == 1:
                                fill.pop(0)()
                            if len(pend) > DEPTH:
                                ci0, c0, p0 = pend.pop(0)
                                emit_pv(hp, ctx, len(chunks), ci0, c0, p0)
                        if deferred_epi is not None:
                            deferred_epi()
                            deferred_epi = None
                        while fill:
                            fill.pop(0)()
                        for ci0, c0, p0 in pend:
                            emit_pv(hp, ctx, len(chunks), ci0, c0, p0)
                        deferred_epi = make_epilogue(hp, q0, ctx)
                deferred_epi()
                attn_ctx.close()

    nc.compile()
    return nc



# ---------------------------------------------------------------------------
# host-side data prep
# ---------------------------------------------------------------------------

def _perm_cols():
    perm = np.empty(3 * D, dtype=np.int64)
    for j in range(3):
        for h in range(NH):
            for d in range(HS):
                perm[j * D + h * HS + d] = j * D + d * NH + h
    return perm


def _core_inputs(xT, W2, b2, B2, HG):
    cst, jk, r0k, jv, r0v = _phase(B2)

    def xt_slice(c, ncols, guard=0):
        vs = cst[c]["vstart"] - guard
        sl = np.zeros((D, ncols), dtype=np.float32)
        lo, hi = max(0, vs), min(B * T, vs + ncols)
        sl[:, lo - vs: hi - vs] = xT[:, lo:hi]
        return sl

    WQK = np.empty((D, 768), dtype=np.float32)
    BQKf = np.empty(768, dtype=np.float32)
    for jj in range(3):
        src = jj * D + HG * 256
        WQK[:, jj * 256:(jj + 1) * 256] = W2[:, src:src + 256]
        BQKf[jj * 256:(jj + 1) * 256] = b2[src:src + 256]
    BQK = BQKf.reshape(6, 128).T.copy()  # [128, 6]: col fc, partition p

    WV = np.zeros((D, 780), dtype=np.float32)
    BV = np.zeros((1, 780), dtype=np.float32)
    for jj in range(3):
        for hl in range(4):
            src = jj * D + HG * 256 + hl * 64
            cb = (jj * 4 + hl) * 65
            WV[:, cb:cb + 64] = W2[:, src:src + 64]
            BV[0, cb:cb + 64] = b2[src:src + 64]
            BV[0, cb + 64] = 1.0

    return {
        "XTQ": xt_slice(0, 768),
        "XTK": xt_slice(1, 768),
        "XTV": xt_slice(2, NVV, guard=GUARD),
        "WQK": WQK,
        "WV": WV,
        "BQK": np.ascontiguousarray(BQK),
        "BV": BV,
        "ONES": np.ones((1, 128), dtype=np.float32),
        "ZEROS": np.zeros((128, 260), dtype=np.float32),
    }


# ---------------------------------------------------------------------------
# concurrent two-program dispatch (4+4 cores)
# ---------------------------------------------------------------------------

def _sharded_fn(nc, dev_lo, dev_hi):
    import jax
    from jax.sharding import Mesh, PartitionSpec
    from jax.experimental.shard_map import shard_map
    from concourse import bass2jax
    from concourse.bass2jax import _bass_exec_p, install_neuronx_cc_hook

    install_neuronx_cc_hook()
    n_cores = dev_hi - dev_lo

    in_names, out_names, out_avals, zero_shapes = [], [], [], []
    partition_name = (
        nc.partition_id_tensor.name if nc.partition_id_tensor else None
    )
    for alloc in nc.m.functions[0].allocations:
        if not isinstance(alloc, mybir.MemoryLocationSet):
            continue
        name = alloc.memorylocations[0].name
        if alloc.kind == "ExternalInput":
            if name != partition_name:
                in_names.append(name)
        elif alloc.kind == "ExternalOutput":
            np_dt = mybir.dt.np(alloc.dtype)
            out_avals.append(
                jax.core.ShapedArray(tuple(alloc.tensor_shape), np_dt)
            )
            out_names.append(name)
            zero_shapes.append((tuple(alloc.tensor_shape), np_dt))
    n_params = len(in_names)
    all_in_names = list(in_names) + list(out_names)
    if partition_name is not None:
        all_in_names.append(partition_name)

    donate = tuple(range(n_params, n_params + len(out_names)))

    def _body(*args):
        operands = list(args)
        if partition_name is not None:
            operands.append(bass2jax.partition_id_tensor())
        outs = _bass_exec_p.bind(
            *operands,
            out_avals=tuple(out_avals),
            in_names=tuple(all_in_names),
            out_names=tuple(out_names),
            lowering_input_output_aliases=(),
            sim_require_finite=True,
            sim_require_nnan=True,
            nc=nc,
        )
        return tuple(outs)

    devices = jax.devices()[dev_lo:dev_hi]
    mesh = Mesh(np.asarray(devices), ("core",))
    in_specs = (PartitionSpec("core"),) * (n_params + len(out_names))
    out_specs = (PartitionSpec("core"),) * len(out_names)
    fn = jax.jit(
        shard_map(_body, mesh=mesh, in_specs=in_specs, out_specs=out_specs,
                  check_rep=False),
        donate_argnums=donate,
        keep_unused=True,
    )
    return fn, in_names, out_names, out_avals, zero_shapes, n_cores


def _concat_inputs(in_maps, in_names):
    return [
        np.concatenate([np.asarray(m[name]) for m in in_maps], axis=0)
        for name in in_names
    ]


def kernel(x, W_qkv, b_qkv):
    x = np.asarray(x, dtype=np.float32)
    W_qkv = np.asarray(W_qkv, dtype=np.float32)
    b_qkv = np.asarray(b_qkv, dtype=np.float32)

    if "progs" not in _CACHE:
        _CACHE["progs"] = {
            B2: _build_program(B2, repeat=int(os.environ.get("KREPEAT", "1")))
            for B2 in range(2)
        }
        _CACHE["fns"] = {
            0: _sharded_fn(_CACHE["progs"][0], 0, 4),
            1: _sharded_fn(_CACHE["progs"][1], 4, 8),
        }

    perm = _perm_cols()
    W2 = W_qkv[:, perm]
    b2 = b_qkv[perm]
    xT = np.ascontiguousarray(x.reshape(B * T, D).T)

    results = {}
    pending = []
    for B2 in range(2):
        fn, in_names, out_names, out_avals, zero_shapes, n_cores = _CACHE["fns"][B2]
        in_maps = [_core_inputs(xT, W2, b2, B2, HG) for HG in range(4)]
        concat_in = _concat_inputs(in_maps, in_names)
        concat_zeros = [
            np.zeros((n_cores * s[0], *s[1:]), d) for (s, d) in zero_shapes
        ]
        out_arrs = fn(*concat_in, *concat_zeros)  # async dispatch
        pending.append((B2, out_names, out_avals, n_cores, out_arrs))

    out_full = np.zeros((B, T, D), dtype=np.float32)
    for B2, out_names, out_avals, n_cores, out_arrs in pending:
        per_core = np.asarray(out_arrs[0]).reshape(n_cores, T, 256)
        for HG in range(4):
            out_full[B2, :, HG * 256:(HG + 1) * 256] = per_core[HG]
    return out_full



# revision 12
# speedup vs baseline: 1.2226x; 1.2226x over previous
"""Trainium2 Bass kernel for nn_CausalAttentionKVCache (B=2, T=2048, D=1024, 16 heads).

Sharding: 8 cores = 2 batch-halves x 4 head-groups (4 heads each).
Two compiled SPMD programs (one per batch-half, phase constants differ mod 3),
dispatched concurrently on jax devices [0:4] and [4:8].

The module's reshape y.view(3,B,T,hs,nh) scrambles tokens: flat row
v = (c*B*T + b*T + t)//3 of y=[x@W+b] holds token t of tensor c (q/k/v) in
column block j=(c*B*T+b*T+t)%3. With a host-side column permutation of W
(W2[:, j*1024+h*64+d] = W[:, j*1024+d*16+h]) each head's 64 features are
contiguous, and q/k/v become contiguous ~683-row bands of y differing only in
which x^T column slice feeds the matmul — so ONE weight load serves all three
projections.

All inputs stream in bf16 (halves DMA; matmul cost on TRN2 is per output
column regardless of dtype). Q^T/K^T/V^T are projected features-on-partitions
and descrambled into token-order SBUF tiles via strided PSUM evictions. V^T is
then PE-transposed per 128-token chunk into token-on-partitions yvs (with a
ones column per head for the softmax denominator), so attention k-chunks are
128 CONSECUTIVE tokens: the causal staircase per chunk is only 128 wide
(vs 384 for v-row chunks), cutting S/PV/exp work ~15%. Attention runs
S^T = K^T.T@Q^T (k on partitions), exp on ScalarE (scale=1/8 fused, no
max-subtraction: scores ~ N(0,1)), diagonal triangles zeroed by gpsimd
affine_select, PV with V stationary accumulating ctx^T, PE-transpose +
per-partition reciprocal for the final division (f32 epilogue).
"""
import sys
import os

sys.path.insert(0, "/opt/trn_rl_repo")

import numpy as np

import concourse.bass as bass
import concourse.bacc as bacc
import concourse.mybir as mybir
import concourse.tile as tile
from concourse.masks import make_identity

B, T, D, NH, HS = 2, 2048, 1024, 16, 64
NV2 = 684         # v-rows per (tensor, batch-half) band (padded)
QW = 512          # q window
NCH = 16          # k/v chunks of 128 tokens
F32 = mybir.dt.float32
BF16 = mybir.dt.bfloat16

_CACHE = {}


def _phase(B2):
    """Compile-time residue/offset constants for batch-half B2."""
    cst = {}
    for c in range(3):
        u0 = c * B * T + B2 * T
        vstart = u0 // 3
        rc_of_jj, r0_of_jj = {}, {}
        for rc in range(3):
            jj = (u0 + rc) % 3
            rc_of_jj[jj] = rc
            r0_of_jj[jj] = (u0 + rc - jj) // 3 - vstart
        cst[c] = dict(u0=u0, vstart=vstart, rc=rc_of_jj, r0=r0_of_jj)
    return cst


def _nrc(rc):
    return 683 if rc < 2 else 682


def _build_program(B2, repeat=1):
    cst = _phase(B2)
    nc = bacc.Bacc("TRN2", target_bir_lowering=False, debug=False, num_devices=4)

    xtq_d = nc.dram_tensor("XTQ", [D, NV2], BF16, kind="ExternalInput")
    xtk_d = nc.dram_tensor("XTK", [D, NV2], BF16, kind="ExternalInput")
    xtv_d = nc.dram_tensor("XTV", [D, NV2], BF16, kind="ExternalInput")
    wqk_d = nc.dram_tensor("WQK", [D, 768], BF16, kind="ExternalInput")
    bqk_d = nc.dram_tensor("BQK", [128, 6], F32, kind="ExternalInput")
    out_d = nc.dram_tensor("OUT", [T, 256], F32, kind="ExternalOutput")

    with tile.TileContext(nc) as tc:
        with (
            tc.tile_pool(name="const", bufs=1) as constp,
            tc.tile_pool(name="wpool", bufs=1) as wpool,
            tc.tile_pool(name="xpool", bufs=1) as xpool,
            tc.tile_pool(name="qkv", bufs=1) as qkvp,
            tc.tile_pool(name="ppool", bufs=6) as ppool,
            tc.tile_pool(name="stashp", bufs=24) as stashp,
            tc.tile_pool(name="cxpool", bufs=2) as cxpool,
            tc.tile_pool(name="opool", bufs=2) as opool,
            tc.tile_pool(name="rpool", bufs=2) as rpool,
        ):
            from contextlib import ExitStack
            identity_b = constp.tile([128, 128], BF16)
            make_identity(nc, identity_b[:])
            identity_f = constp.tile([128, 128], F32)
            make_identity(nc, identity_f[:])

            wqk = wpool.tile([128, 2, 8, 384], BF16)
            bqk = wpool.tile([128, 6], F32)
            # hp0 weight half first (one DMA) so Q{hp0} can start ~3.5us in
            wqk_r = wqk_d.rearrange("(c p) (h f) -> p c h f", p=128, h=2)
            nc.sync.dma_start(wqk[:, 0, 0:4, :], wqk_r[:, 0:4, 0, :])
            nc.sync.dma_start(wqk[:, 0, 4:8, :], wqk_r[:, 4:8, 0, :])

            def wslice(ic, fc):
                # weight cols for fc=(jj,hp): host layout groups hp halves
                jj, hp = fc // 2, fc % 2
                return wqk[:, hp, ic, jj * 128:(jj + 1) * 128]

            for _rep in range(repeat):
                proj_ctx = ExitStack()
                psqk = proj_ctx.enter_context(
                    tc.tile_pool(name="psqk", bufs=6, space="PSUM"))
                pstr = proj_ctx.enter_context(
                    tc.tile_pool(name="pstr", bufs=2, space="PSUM"))
                qt = qkvp.tile([128, 2, T], BF16, tag="qt")
                kt = qkvp.tile([128, 2, T], BF16, tag="kt")
                vt = qkvp.tile([128, 2, T], BF16, tag="vt")
                yvs = qkvp.tile([128, NCH, 2, 130], BF16, tag="yvs")
                nc.vector.memset(yvs[:, :, :, 64:130:65], 1.0)

                xts = {}
                for si, (src_d, nm) in enumerate(
                        [(xtq_d, "xq"), (xtk_d, "xk"), (xtv_d, "xv")]):
                    xts[si] = xpool.tile([128, 8, NV2], BF16, tag=nm, name=nm)

                def load_x(si, src_d, split):
                    src = src_d.rearrange("(c p) v -> p c v", p=128)
                    if split:
                        for ic in range(8):
                            nc.sync.dma_start(xts[si][:, ic, :], src[:, ic, :])
                    else:
                        nc.sync.dma_start(xts[si][:], src[:, :, :])

                load_x(0, xtq_d, True)
                nc.sync.dma_start(bqk[:], bqk_d[:, :])
                load_x(1, xtk_d, False)
                load_x(2, xtv_d, True)
                nc.sync.dma_start(wqk[:, 1, :, :], wqk_r[:, :, 1, :])

                SPANS = ((0, 512), (512, NV2))

                def evict_span(si, fc, sp, ps):
                    jj, hp = fc // 2, fc % 2
                    rc, r0 = cst[si]["rc"][jj], cst[si]["r0"][jj]
                    n = _nrc(rc)
                    dst = (qt, kt, vt)[si]
                    if sp == 0:
                        lo, cnt, t0 = r0, 512 - r0, rc
                    else:
                        lo, cnt, t0 = 0, r0 + n - 512, rc + 3 * (512 - r0)
                    nc.vector.tensor_scalar_add(
                        dst[:, hp, t0: t0 + 3 * (cnt - 1) + 1: 3],
                        ps[:, lo: lo + cnt],
                        bqk[:, fc: fc + 1],
                    )

                def emit_span(si, fc, sp, pool=None, tag="g"):
                    """Project 128 features (block fc) of tensor si over
                    v-row span sp; descramble-evict token-order + bias."""
                    v0, v1 = SPANS[sp]
                    ps = (pool or psqk).tile([128, 512], F32, tag=tag,
                                             name="psg")
                    for ic in range(8):
                        nc.tensor.matmul(
                            ps[:, 0:v1 - v0],
                            wslice(ic, fc),
                            xts[si][:, ic, v0:v1],
                            start=(ic == 0),
                            stop=(ic == 7),
                        )
                    evict_span(si, fc, sp, ps)

                def emit_triple_span(si, fcs, sp):
                    """ic-innermost across 3 psum groups: matmuls consume
                    each x/w DMA slice as it lands."""
                    v0, v1 = SPANS[sp]
                    pss_ = [psqk.tile([128, 512], F32, tag="g", name="psg")
                            for _ in fcs]
                    for ic in range(8):
                        for gi, fc in enumerate(fcs):
                            nc.tensor.matmul(
                                pss_[gi][:, 0:v1 - v0],
                                wslice(ic, fc),
                                xts[si][:, ic, v0:v1],
                                start=(ic == 0),
                                stop=(ic == 7),
                            )
                    for gi, fc in enumerate(fcs):
                        evict_span(si, fc, sp, pss_[gi])

                def emit_vtr(hp, c, pool=None, tag="pstr"):
                    """Transpose V^T chunk c (128 tokens) to token-rows, into
                    yvs with per-head ones columns left intact."""
                    tr = (pool or pstr).tile([128, 128], BF16, tag=tag,
                                             name="vtr",
                                             padded_shape=[128, 1024])
                    nc.tensor.transpose(
                        tr[:], vt[:, hp, 128 * c: 128 * (c + 1)],
                        identity_b[:, :])
                    for hr in range(2):
                        nc.vector.tensor_copy(
                            yvs[:, c, hp, hr * 65: hr * 65 + 64],
                            tr[:, hr * 64:(hr + 1) * 64])

                # ---- lead-in: hp0 projections span0 (ic-interleaved),
                # first hp0 V transposes; the rest stream in as fillers ----
                emit_triple_span(0, (0, 2, 4), 0)
                emit_triple_span(1, (0, 2, 4), 0)
                emit_triple_span(2, (0, 2, 4), 0)
                for c in range(4):
                    emit_vtr(0, c)
                proj_ctx.close()

                attn_ctx = ExitStack()
                pss = attn_ctx.enter_context(
                    tc.tile_pool(name="pss", bufs=3, space="PSUM"))
                psctx = attn_ctx.enter_context(
                    tc.tile_pool(name="psctx", bufs=2, space="PSUM"))

                # ---- attention emitters ----
                def emit_s_exp(hp, q0, c, ppool_=None):
                    a = max(0, 128 * c - q0)
                    s_ps = pss.tile([128, 2, QW], F32, tag="s", name="s_ps")
                    for hr in range(2):
                        pr = slice(hr * 64, hr * 64 + 64)
                        nc.tensor.matmul(
                            s_ps[:, hr, a:QW],
                            kt[pr, hp, 128 * c: 128 * (c + 1)],
                            qt[pr, hp, q0 + a: q0 + QW],
                            start=True,
                            stop=True,
                            tile_position=(hr * 64, 0),
                        )
                    p_sb = (ppool_ or ppool).tile([128, 2, QW], BF16,
                                                  tag="p", name="p_sb")
                    nc.scalar.activation(
                        p_sb[:, :, a:QW],
                        s_ps[:, :, a:QW],
                        mybir.ActivationFunctionType.Exp,
                        scale=float(HS) ** -0.5,
                    )
                    if 128 * c >= q0:   # diagonal chunk: zero upper triangle
                        nc.gpsimd.affine_select(
                            out=p_sb[:, :, a:a + 128],
                            in_=p_sb[:, :, a:a + 128],
                            pattern=[[0, 2], [1, 128]],
                            compare_op=mybir.AluOpType.is_ge,
                            fill=0.0,
                            base=0,
                            channel_multiplier=-1,
                        )
                    return a, p_sb

                def emit_pv(hp, ctx, nchunks, ci, c, a, p_sb):
                    for hr in range(2):
                        nc.tensor.matmul(
                            ctx[hr][:, a:QW],
                            yvs[:, c, hp, hr * 65:(hr + 1) * 65],
                            p_sb[:, hr, a:QW],
                            start=(ci == 0),
                            stop=(ci == nchunks - 1),
                        )

                def make_epilogue(hp, q0, ctx):
                    def epi():
                        cx = cxpool.tile([65, 2, QW], F32, tag="cx", name="cx")
                        for hr in range(2):
                            nc.vector.tensor_copy(cx[:, hr, :], ctx[hr][:])
                        o_sb = opool.tile([128, 4, 2, 64], F32, tag="o",
                                          name="o_sb")
                        for hr in range(2):
                            for qb in range(QW // 128):
                                tr = pss.tile([128, 65], F32, tag="s",
                                              name="tr")
                                nc.tensor.transpose(
                                    tr[:],
                                    cx[0:65, hr, qb * 128:(qb + 1) * 128],
                                    identity_f[0:65, 0:65],
                                )
                                rec = rpool.tile([128, 1], F32, tag="rec",
                                                 name="rec")
                                nc.vector.reciprocal(rec[:], tr[:, 64:65])
                                nc.vector.tensor_scalar_mul(
                                    o_sb[:, qb, hr, :], tr[:, 0:64], rec[:]
                                )
                        nc.sync.dma_start(
                            out_d[q0: q0 + QW,
                                  hp * 128:(hp + 1) * 128].rearrange(
                                "(qb p) d -> p qb d", p=128),
                            o_sb[:],
                        )
                    return epi

                # filler schedule: remaining projection spans + V
                # transposes, placed so each window's filler PE work roughly
                # covers its exp (Act) surplus and all deps precede use
                def f_sp(si, fc, sp):
                    return lambda: emit_span(si, fc, sp, pool=pss, tag="s")

                def f_sp3(sis, fcs, sp):
                    return lambda: [emit_span(si, fc, sp, pool=pss, tag="s")
                                    for si, fc in zip(sis, fcs)]

                def f_tr(hp, cs):
                    return lambda: [emit_vtr(1 if hp else 0, c, pool=pss,
                                             tag="s") for c in cs]

                STASH = {}

                def f_stash(hp, qi, cs):
                    def go():
                        for c in cs:
                            STASH[(hp, qi, c)] = emit_s_exp(
                                hp, QW * qi, c, ppool_=stashp)
                    return go

                fills = {
                    (0, 0): [f_sp(2, 0, 1), f_sp(2, 2, 1), f_sp(2, 4, 1),
                             f_sp(0, 0, 1), f_sp(0, 2, 1)],
                    (0, 1): [f_tr(0, (4, 5)), f_tr(0, (6, 7)),
                             f_sp(0, 4, 1), f_sp(1, 0, 1), f_sp(1, 2, 1),
                             f_sp(1, 4, 1), f_sp(0, 1, 0), f_tr(0, (8, 9))],
                    (0, 2): [f_tr(0, (10, 11)), f_tr(0, (12, 13)),
                             f_tr(0, (14, 15)), f_sp(2, 1, 0),
                             f_sp(1, 1, 0), f_sp(0, 3, 0),
                             f_stash(0, 3, (0, 1)), f_stash(0, 3, (2, 3))],
                    (0, 3): [f_sp(2, 3, 0), f_sp(1, 3, 0), f_sp(0, 5, 0),
                             f_sp(2, 5, 0), f_sp(1, 5, 0),
                             f_tr(1, (0, 1)), f_tr(1, (2, 3))],
                    (1, 0): [f_sp(0, 1, 1), f_sp(0, 3, 1), f_sp(0, 5, 1),
                             f_sp(2, 1, 1), f_sp(2, 3, 1), f_sp(2, 5, 1),
                             f_tr(1, (4, 5)), f_stash(1, 3, (0, 1))],
                    (1, 1): [f_stash(1, 3, (2, 3)), f_sp(1, 1, 1),
                             f_sp(1, 3, 1), f_sp(1, 5, 1), f_tr(1, (6, 7)),
                             f_tr(1, (8, 9)), f_tr(1, (10,)),
                             f_stash(1, 3, (4, 5)), f_stash(1, 3, (6, 7))],
                    (1, 2): [f_tr(1, (11,)), f_tr(1, (12, 13)),
                             f_tr(1, (14, 15)), f_stash(1, 3, (8, 9)),
                             f_stash(1, 3, (10, 11))],
                }

                DEPTH = int(os.environ.get("KDEPTH", "3"))
                deferred_epi = None
                for hp in range(2):
                    for qi, q0 in enumerate(range(0, T, QW)):
                        nchunks = min(NCH, 4 * qi + 4)
                        fill = list(fills.get((hp, qi), []))
                        ctx = [
                            psctx.tile([65, QW], F32, tag="ctx",
                                       name=f"ctx{hr}")
                            for hr in range(2)
                        ]
                        order = list(range(nchunks))
                        if any((hp, qi, c) in STASH for c in range(nchunks)):
                            order = (list(range(4 * qi, nchunks))
                                     + list(range(4 * qi)))
                        pend = []
                        for ci, c in enumerate(order):
                            if (hp, qi, c) in STASH:
                                pend.append((ci, c,
                                             *STASH.pop((hp, qi, c))))
                            else:
                                pend.append((ci, c,
                                             *emit_s_exp(hp, q0, c)))
                            if ci == int(os.environ.get("KEPI", "4")) and deferred_epi is not None:
                                deferred_epi()
                                deferred_epi = None
                            if fill:
                                fill.pop(0)()
                            if len(pend) > DEPTH:
                                ci0, c0, a0, p0 = pend.pop(0)
                                emit_pv(hp, ctx, nchunks, ci0, c0, a0, p0)
                        if deferred_epi is not None:
                            deferred_epi()
                            deferred_epi = None
                        while fill:
                            fill.pop(0)()
                        for ci0, c0, a0, p0 in pend:
                            emit_pv(hp, ctx, nchunks, ci0, c0, a0, p0)
                        deferred_epi = make_epilogue(hp, q0, ctx)
                deferred_epi()
                attn_ctx.close()

    nc.compile()
    return nc


# ---------------------------------------------------------------------------
# host-side data prep
# ---------------------------------------------------------------------------

def _perm_cols():
    perm = np.empty(3 * D, dtype=np.int64)
    for j in range(3):
        for h in range(NH):
            for d in range(HS):
                perm[j * D + h * HS + d] = j * D + d * NH + h
    return perm


def _core_inputs(xT, W2, b2, B2, HG):
    import ml_dtypes
    bf16 = ml_dtypes.bfloat16
    cst = _phase(B2)

    def xt_slice(c):
        vs = cst[c]["vstart"]
        sl = np.zeros((D, NV2), dtype=bf16)
        lo, hi = max(0, vs), min(B * T, vs + NV2)
        sl[:, lo - vs: hi - vs] = xT[:, lo:hi].astype(bf16)
        return sl

    # weight layout: [D, hp, jj, 128]: hp halves contiguous so the hp0 half
    # can stream first
    WQK = np.empty((D, 2, 3, 128), dtype=bf16)
    BQKf = np.empty(768, dtype=np.float32)
    for jj in range(3):
        for hp in range(2):
            src = jj * D + HG * 256 + hp * 128
            WQK[:, hp, jj, :] = W2[:, src:src + 128].astype(bf16)
            fc = jj * 2 + hp
            BQKf[fc * 128:(fc + 1) * 128] = b2[src:src + 128]
    BQK = BQKf.reshape(6, 128).T.copy()  # [128, 6]: col fc, partition p

    return {
        "XTQ": xt_slice(0),
        "XTK": xt_slice(1),
        "XTV": xt_slice(2),
        "WQK": np.ascontiguousarray(WQK.reshape(D, 768)),
        "BQK": np.ascontiguousarray(BQK),
    }


# ---------------------------------------------------------------------------
# concurrent two-program dispatch (4+4 cores)
# ---------------------------------------------------------------------------

def _sharded_fn(nc, dev_lo, dev_hi):
    import jax
    from jax.sharding import Mesh, PartitionSpec
    from jax.experimental.shard_map import shard_map
    from concourse import bass2jax
    from concourse.bass2jax import _bass_exec_p, install_neuronx_cc_hook

    install_neuronx_cc_hook()
    n_cores = dev_hi - dev_lo

    in_names, out_names, out_avals, zero_shapes = [], [], [], []
    partition_name = (
        nc.partition_id_tensor.name if nc.partition_id_tensor else None
    )
    for alloc in nc.m.functions[0].allocations:
        if not isinstance(alloc, mybir.MemoryLocationSet):
            continue
        name = alloc.memorylocations[0].name
        if alloc.kind == "ExternalInput":
            if name != partition_name:
                in_names.append(name)
        elif alloc.kind == "ExternalOutput":
            np_dt = mybir.dt.np(alloc.dtype)
            out_avals.append(
                jax.core.ShapedArray(tuple(alloc.tensor_shape), np_dt)
            )
            out_names.append(name)
            zero_shapes.append((tuple(alloc.tensor_shape), np_dt))
    n_params = len(in_names)
    all_in_names = list(in_names) + list(out_names)
    if partition_name is not None:
        all_in_names.append(partition_name)

    donate = tuple(range(n_params, n_params + len(out_names)))

    def _body(*args):
        operands = list(args)
        if partition_name is not None:
            operands.append(bass2jax.partition_id_tensor())
        outs = _bass_exec_p.bind(
            *operands,
            out_avals=tuple(out_avals),
            in_names=tuple(all_in_names),
            out_names=tuple(out_names),
            lowering_input_output_aliases=(),
            sim_require_finite=True,
            sim_require_nnan=True,
            nc=nc,
        )
        return tuple(outs)

    devices = jax.devices()[dev_lo:dev_hi]
    mesh = Mesh(np.asarray(devices), ("core",))
    in_specs = (PartitionSpec("core"),) * (n_params + len(out_names))
    out_specs = (PartitionSpec("core"),) * len(out_names)
    fn = jax.jit(
        shard_map(_body, mesh=mesh, in_specs=in_specs, out_specs=out_specs,
                  check_rep=False),
        donate_argnums=donate,
        keep_unused=True,
    )
    return fn, in_names, out_names, out_avals, zero_shapes, n_cores


def _concat_inputs(in_maps, in_names):
    return [
        np.concatenate([np.asarray(m[name]) for m in in_maps], axis=0)
        for name in in_names
    ]


def kernel(x, W_qkv, b_qkv):
    x = np.asarray(x, dtype=np.float32)
    W_qkv = np.asarray(W_qkv, dtype=np.float32)
    b_qkv = np.asarray(b_qkv, dtype=np.float32)

    if "progs" not in _CACHE:
        _CACHE["progs"] = {
            B2: _build_program(B2, repeat=int(os.environ.get("KREPEAT", "1")))
            for B2 in range(2)
        }
        _CACHE["fns"] = {
            0: _sharded_fn(_CACHE["progs"][0], 0, 4),
            1: _sharded_fn(_CACHE["progs"][1], 4, 8),
        }

    perm = _perm_cols()
    W2 = W_qkv[:, perm]
    b2 = b_qkv[perm]
    xT = np.ascontiguousarray(x.reshape(B * T, D).T)

    pending = []
    for B2 in range(2):
        fn, in_names, out_names, out_avals, zero_shapes, n_cores = _CACHE["fns"][B2]
        in_maps = [_core_inputs(xT, W2, b2, B2, HG) for HG in range(4)]
        concat_in = _concat_inputs(in_maps, in_names)
        concat_zeros = [
            np.zeros((n_cores * s[0], *s[1:]), d) for (s, d) in zero_shapes
        ]
        out_arrs = fn(*concat_in, *concat_zeros)  # async dispatch
        pending.append((B2, out_names, out_avals, n_cores, out_arrs))

    out_full = np.zeros((B, T, D), dtype=np.float32)
    for B2, out_names, out_avals, n_cores, out_arrs in pending:
        per_core = np.asarray(out_arrs[0]).reshape(n_cores, T, 256)
        for HG in range(4):
            out_full[B2, :, HG * 256:(HG + 1) * 256] = per_core[HG]
    return out_full


# revision 38
# speedup vs baseline: 1.3096x; 1.0711x over previous
"""Trainium2 Bass kernel for nn_CausalAttentionKVCache (B=2, T=2048, D=1024, 16 heads).

Sharding: 8 cores = 2 batch-halves x 4 head-groups (4 heads each).
Two compiled SPMD programs (one per batch-half, phase constants differ mod 3),
dispatched concurrently on jax devices [0:4] and [4:8].

The module's reshape y.view(3,B,T,hs,nh) scrambles tokens: flat row
v = (c*B*T + b*T + t)//3 of y=[x@W+b] holds token t of tensor c (q/k/v) in
column block j=(c*B*T+b*T+t)%3. With a host-side column permutation of W
(W2[:, j*1024+h*64+d] = W[:, j*1024+d*16+h]) each head's 64 features are
contiguous, and q/k/v become contiguous ~683-row bands of y differing only in
which x^T column slice feeds the matmul -- so ONE weight load serves all three
projections.

All inputs stream in bf16 (halves DMA; TRN2 matmul cost is per output column
regardless of dtype, and bf16 lifts the f32r >=256-column constraint).
Q^T/K^T/V^T are projected features-on-partitions in two v-row spans
(0,512),(512,684) per 128-feature group -- separately schedulable PSUM
accumulation groups -- then descramble-evicted (stride-3 DVE writes) into
token-order SBUF tiles with fused bias. V^T is PE-transposed per 128-token
chunk into token-on-partitions yvs (ones column per head for the softmax
denominator), so attention k-chunks are 128 CONSECUTIVE tokens: the causal
staircase per chunk is 128 wide (vs 384 for v-row chunks), cutting S/PV/exp
work ~15%. Attention runs S^T = K^T.T@Q^T (k on partitions, two heads via PE
quadrants), exp on ScalarE (scale=1/8 fused; scores ~ N(0,1) so no
max-subtraction), diagonal triangles zeroed post-exp by gpsimd affine_select,
PV with V stationary accumulating ctx^T in PSUM, PE-transpose +
per-partition reciprocal for the final division (f32 epilogue).

Schedule: lead-in covers only span-0 hp0 projections (ic-interleaved with the
per-ic DMA stream); all remaining projection spans and V transposes are
fillers popped one per attention chunk, their window placement tuned by
randomized hill-climb against the TimelineSim cost model so the ScalarE exp
stream never starves the PE. Cost-model 141.1us/core; measured rel err 4.6e-3.
"""
import sys
import os

sys.path.insert(0, "/opt/trn_rl_repo")

import numpy as np

import concourse.bass as bass
import concourse.bacc as bacc
import concourse.mybir as mybir
import concourse.tile as tile
from concourse.masks import make_identity

B, T, D, NH, HS = 2, 2048, 1024, 16, 64
NV2 = 684         # v-rows per (tensor, batch-half) band (padded)
QW = 512          # q window
NCH = 16          # k/v chunks of 128 tokens
F32 = mybir.dt.float32
BF16 = mybir.dt.bfloat16

_CACHE = {}


def _phase(B2):
    """Compile-time residue/offset constants for batch-half B2."""
    cst = {}
    for c in range(3):
        u0 = c * B * T + B2 * T
        vstart = u0 // 3
        rc_of_jj, r0_of_jj = {}, {}
        for rc in range(3):
            jj = (u0 + rc) % 3
            rc_of_jj[jj] = rc
            r0_of_jj[jj] = (u0 + rc - jj) // 3 - vstart
        cst[c] = dict(u0=u0, vstart=vstart, rc=rc_of_jj, r0=r0_of_jj)
    return cst


def _nrc(rc):
    return 683 if rc < 2 else 682


def _build_program(B2, repeat=1):
    cst = _phase(B2)
    nc = bacc.Bacc("TRN2", target_bir_lowering=False, debug=False, num_devices=4)

    xtq_d = nc.dram_tensor("XTQ", [D, NV2], BF16, kind="ExternalInput")
    xtk_d = nc.dram_tensor("XTK", [D, NV2], BF16, kind="ExternalInput")
    xtv_d = nc.dram_tensor("XTV", [D, NV2], BF16, kind="ExternalInput")
    wqk_d = nc.dram_tensor("WQK", [D, 768], BF16, kind="ExternalInput")
    bqk_d = nc.dram_tensor("BQK", [128, 6], F32, kind="ExternalInput")
    out_d = nc.dram_tensor("OUT", [T, 256], F32, kind="ExternalOutput")

    with tile.TileContext(nc) as tc:
        with (
            tc.tile_pool(name="const", bufs=1) as constp,
            tc.tile_pool(name="wpool", bufs=1) as wpool,
            tc.tile_pool(name="xpool", bufs=1) as xpool,
            tc.tile_pool(name="qkv", bufs=1) as qkvp,
            tc.tile_pool(name="ppool", bufs=14) as ppool,
            tc.tile_pool(name="stashp", bufs=24) as stashp,
            tc.tile_pool(name="cxpool", bufs=6) as cxpool,
            tc.tile_pool(name="opool", bufs=4) as opool,
            tc.tile_pool(name="rpool", bufs=8) as rpool,
        ):
            from contextlib import ExitStack
            identity_b = constp.tile([128, 128], BF16)
            make_identity(nc, identity_b[:])
            identity_f = constp.tile([128, 128], F32)
            make_identity(nc, identity_f[:])

            wqk = wpool.tile([128, 2, 8, 384], BF16)
            bqk = wpool.tile([128, 6], F32)
            # hp0 weight half first (one DMA) so Q{hp0} can start ~3.5us in
            wqk_r = wqk_d.rearrange("(c p) (h f) -> p c h f", p=128, h=2)
            nc.sync.dma_start(wqk[:, 0, 0:4, :], wqk_r[:, 0:4, 0, :])
            nc.sync.dma_start(wqk[:, 0, 4:8, :], wqk_r[:, 4:8, 0, :])

            def wslice(ic, fc):
                # weight cols for fc=(jj,hp): host layout groups hp halves
                jj, hp = fc // 2, fc % 2
                return wqk[:, hp, ic, jj * 128:(jj + 1) * 128]

            for _rep in range(repeat):
                proj_ctx = ExitStack()
                psqk = proj_ctx.enter_context(
                    tc.tile_pool(name="psqk", bufs=6, space="PSUM"))
                pstr = proj_ctx.enter_context(
                    tc.tile_pool(name="pstr", bufs=2, space="PSUM"))
                qt = qkvp.tile([128, 2, T], BF16, tag="qt")
                kt = qkvp.tile([128, 2, T], BF16, tag="kt")
                vt = qkvp.tile([128, 2, T], BF16, tag="vt")
                yvs = qkvp.tile([128, NCH, 2, 130], BF16, tag="yvs")
                nc.vector.memset(yvs[:, :, :, 64:130:65], 1.0)

                xts = {}
                for si, (src_d, nm) in enumerate(
                        [(xtq_d, "xq"), (xtk_d, "xk"), (xtv_d, "xv")]):
                    xts[si] = xpool.tile([128, 8, NV2], BF16, tag=nm, name=nm)

                def load_x_sp0(si, src_d, split):
                    src = src_d.rearrange("(c p) v -> p c v", p=128)
                    if split:
                        for ic in range(8):
                            nc.sync.dma_start(xts[si][:, ic, 0:512],
                                              src[:, ic, 0:512])
                    else:
                        nc.sync.dma_start(xts[si][:, :, 0:512],
                                          src[:, :, 0:512])

                def load_x_sp1(si, src_d):
                    src = src_d.rearrange("(c p) v -> p c v", p=128)
                    nc.sync.dma_start(xts[si][:, :, 512:NV2],
                                      src[:, :, 512:NV2])

                # span-0 columns first: they are all the lead-in needs, so
                # attention starts ~3us earlier; span-1 columns stream in
                # under the first attention windows
                load_x_sp0(0, xtq_d, True)
                nc.sync.dma_start(bqk[:], bqk_d[:, :])
                load_x_sp0(2, xtv_d, True)
                load_x_sp0(1, xtk_d, False)
                load_x_sp1(0, xtq_d)
                load_x_sp1(2, xtv_d)
                load_x_sp1(1, xtk_d)
                nc.sync.dma_start(wqk[:, 1, :, :], wqk_r[:, :, 1, :])

                SPANS = ((0, 512), (512, NV2))

                def evict_span(si, fc, sp, ps):
                    jj, hp = fc // 2, fc % 2
                    rc, r0 = cst[si]["rc"][jj], cst[si]["r0"][jj]
                    n = _nrc(rc)
                    dst = (qt, kt, vt)[si]
                    if sp == 0:
                        lo, cnt, t0 = r0, 512 - r0, rc
                    else:
                        lo, cnt, t0 = 0, r0 + n - 512, rc + 3 * (512 - r0)
                    nc.vector.tensor_scalar_add(
                        dst[:, hp, t0: t0 + 3 * (cnt - 1) + 1: 3],
                        ps[:, lo: lo + cnt],
                        bqk[:, fc: fc + 1],
                    )

                def emit_span(si, fc, sp, pool=None, tag="g"):
                    """Project 128 features (block fc) of tensor si over
                    v-row span sp; descramble-evict token-order + bias."""
                    v0, v1 = SPANS[sp]
                    ps = (pool or psqk).tile([128, 512], F32, tag=tag,
                                             name="psg")
                    for ic in range(8):
                        nc.tensor.matmul(
                            ps[:, 0:v1 - v0],
                            wslice(ic, fc),
                            xts[si][:, ic, v0:v1],
                            start=(ic == 0),
                            stop=(ic == 7),
                        )
                    evict_span(si, fc, sp, ps)

                def emit_triple_span(si, fcs, sp):
                    """ic-innermost across 3 psum groups: matmuls consume
                    each x/w DMA slice as it lands."""
                    v0, v1 = SPANS[sp]
                    pss_ = [psqk.tile([128, 512], F32, tag="g", name="psg")
                            for _ in fcs]
                    for ic in range(8):
                        for gi, fc in enumerate(fcs):
                            nc.tensor.matmul(
                                pss_[gi][:, 0:v1 - v0],
                                wslice(ic, fc),
                                xts[si][:, ic, v0:v1],
                                start=(ic == 0),
                                stop=(ic == 7),
                            )
                    for gi, fc in enumerate(fcs):
                        evict_span(si, fc, sp, pss_[gi])

                def emit_vtr(hp, c, pool=None, tag="pstr"):
                    """Transpose V^T chunk c (128 tokens) to token-rows, into
                    yvs with per-head ones columns left intact."""
                    tr = (pool or pstr).tile([128, 128], BF16, tag=tag,
                                             name="vtr",
                                             padded_shape=[128, 1024])
                    nc.tensor.transpose(
                        tr[:], vt[:, hp, 128 * c: 128 * (c + 1)],
                        identity_b[:, :])
                    nc.vector.tensor_copy(
                        yvs[:, c, hp, :].rearrange(
                            "p (h x) -> p h x", h=2)[:, :, 0:64],
                        tr[:].rearrange("p (h x) -> p h x", h=2))

                # ---- lead-in: hp0 projections span0 (ic-interleaved),
                # first hp0 V transposes; the rest stream in as fillers ----
                emit_triple_span(0, (0, 2, 4), 0)
                emit_triple_span(1, (0, 2, 4), 0)
                emit_triple_span(2, (0, 2, 4), 0)
                for c in range(4):
                    emit_vtr(0, c)
                proj_ctx.close()

                attn_ctx = ExitStack()
                pss = attn_ctx.enter_context(
                    tc.tile_pool(name="pss", bufs=3, space="PSUM"))
                psctx = attn_ctx.enter_context(
                    tc.tile_pool(name="psctx", bufs=2, space="PSUM"))

                # ---- attention emitters ----
                def emit_s_exp(hp, q0, c, ppool_=None):
                    a = max(0, 128 * c - q0)
                    s_ps = pss.tile([128, 2, QW], F32, tag="s", name="s_ps")
                    for hr in range(2):
                        pr = slice(hr * 64, hr * 64 + 64)
                        nc.tensor.matmul(
                            s_ps[:, hr, a:QW],
                            kt[pr, hp, 128 * c: 128 * (c + 1)],
                            qt[pr, hp, q0 + a: q0 + QW],
                            start=True,
                            stop=True,
                            tile_position=(hr * 64, 0),
                        )
                    p_sb = (ppool_ or ppool).tile([128, 2, QW], BF16,
                                                  tag="p", name="p_sb")
                    nc.scalar.activation(
                        p_sb[:, :, a:QW],
                        s_ps[:, :, a:QW],
                        mybir.ActivationFunctionType.Exp,
                        scale=float(HS) ** -0.5,
                    )
                    if 128 * c >= q0:   # diagonal chunk: zero upper triangle
                        nc.gpsimd.affine_select(
                            out=p_sb[:, :, a:a + 128],
                            in_=p_sb[:, :, a:a + 128],
                            pattern=[[0, 2], [1, 128]],
                            compare_op=mybir.AluOpType.is_ge,
                            fill=0.0,
                            base=0,
                            channel_multiplier=-1,
                        )
                    return a, p_sb

                def emit_pv(hp, ctx, nchunks, ci, c, a, p_sb):
                    for hr in range(2):
                        nc.tensor.matmul(
                            ctx[hr][:, a:QW],
                            yvs[:, c, hp, hr * 65:(hr + 1) * 65],
                            p_sb[:, hr, a:QW],
                            start=(ci == 0),
                            stop=(ci == nchunks - 1),
                        )

                EPI_BF = os.environ.get("KEPIBF", "0") == "1"
                epi_dt = BF16 if EPI_BF else F32
                epi_id = identity_b if EPI_BF else identity_f

                def make_epilogue(hp, q0, ctx):
                    def epi():
                        cx = cxpool.tile([65, 2, QW], epi_dt, tag="cx",
                                         name="cx")
                        for hr in range(2):
                            nc.vector.tensor_copy(cx[:, hr, :], ctx[hr][:])
                        o_sb = opool.tile([128, 4, 2, 64], F32, tag="o",
                                          name="o_sb")
                        for hr in range(2):
                            for qb in range(QW // 128):
                                tr = pss.tile([128, 65], epi_dt, tag="s",
                                              name="tr")
                                nc.tensor.transpose(
                                    tr[:],
                                    cx[0:65, hr, qb * 128:(qb + 1) * 128],
                                    epi_id[0:65, 0:65],
                                )
                                rec = rpool.tile([128, 1], F32, tag="rec",
                                                 name="rec")
                                nc.vector.reciprocal(rec[:], tr[:, 64:65])
                                nc.vector.tensor_scalar_mul(
                                    o_sb[:, qb, hr, :], tr[:, 0:64], rec[:]
                                )
                        nc.sync.dma_start(
                            out_d[q0: q0 + QW,
                                  hp * 128:(hp + 1) * 128].rearrange(
                                "(qb p) d -> p qb d", p=128),
                            o_sb[:],
                        )
                    return epi

                # filler schedule: remaining projection spans + V
                # transposes, placed so each window's filler PE work roughly
                # covers its exp (Act) surplus and all deps precede use
                def f_sp(si, fc, sp):
                    return lambda: emit_span(si, fc, sp, pool=pss, tag="s")

                def f_sp3(sis, fcs, sp):
                    return lambda: [emit_span(si, fc, sp, pool=pss, tag="s")
                                    for si, fc in zip(sis, fcs)]

                def f_tr(hp, cs):
                    return lambda: [emit_vtr(1 if hp else 0, c, pool=pss,
                                             tag="s") for c in cs]

                STASH = {}

                def f_stash(hp, qi, cs):
                    def go():
                        for c in cs:
                            STASH[(hp, qi, c)] = emit_s_exp(
                                hp, QW * qi, c, ppool_=stashp)
                    return go

                fills = {
                    (0, 0): [f_sp(2, 0, 1), f_sp(2, 2, 1), f_sp(2, 4, 1),
                             f_sp(0, 0, 1), f_sp(0, 2, 1)],
                    (0, 1): [f_tr(0, (4, 5)), f_tr(0, (6, 7)),
                             f_sp(0, 4, 1), f_sp(1, 0, 1), f_sp(1, 2, 1),
                             f_sp(1, 4, 1), f_sp(0, 1, 0), f_tr(0, (8, 9))],
                    (0, 2): [f_tr(0, (10, 11)), f_tr(0, (12, 13)),
                             f_tr(0, (14, 15)), f_sp(2, 1, 0),
                             f_sp(1, 1, 0), f_sp(0, 3, 0)],
                    (0, 3): [f_sp(2, 3, 0), f_sp(1, 3, 0), f_sp(0, 5, 0),
                             f_sp(2, 5, 0), f_sp(1, 5, 0),
                             f_tr(1, (0, 1)), f_tr(1, (2, 3))],
                    (1, 0): [f_sp(2, 1, 1), f_sp(2, 3, 1), f_sp(2, 5, 1),
                             f_sp(0, 1, 1), f_sp(0, 3, 1), f_tr(1, (4, 5))],
                    (1, 1): [f_sp(0, 5, 1), f_sp(1, 1, 1), f_sp(1, 3, 1),
                             f_sp(1, 5, 1), f_tr(1, (6, 7)), f_tr(1, (8, 9)),
                             f_tr(1, (10,))],
                    (1, 2): [f_tr(1, (11,)), f_tr(1, (12, 13)),
                             f_tr(1, (14, 15))],
                }

                DEPTH = int(os.environ.get("KDEPTH", "6"))
                deferred_epi = None
                for hp in range(2):
                    for qi, q0 in enumerate(range(0, T, QW)):
                        nchunks = min(NCH, 4 * qi + 4)
                        fill = list(fills.get((hp, qi), []))
                        ctx = [
                            psctx.tile([65, QW], F32, tag="ctx",
                                       name=f"ctx{hr}")
                            for hr in range(2)
                        ]
                        order = list(range(nchunks))
                        if any((hp, qi, c) in STASH for c in range(nchunks)):
                            order = (list(range(4 * qi, nchunks))
                                     + list(range(4 * qi)))
                        pend = []
                        for ci, c in enumerate(order):
                            if (hp, qi, c) in STASH:
                                pend.append((ci, c,
                                             *STASH.pop((hp, qi, c))))
                            else:
                                pend.append((ci, c,
                                             *emit_s_exp(hp, q0, c)))
                            if ci == int(os.environ.get("KEPI", "8")) and deferred_epi is not None:
                                deferred_epi()
                                deferred_epi = None
                            if fill:
                                fill.pop(0)()
                            if len(pend) > DEPTH:
                                ci0, c0, a0, p0 = pend.pop(0)
                                emit_pv(hp, ctx, nchunks, ci0, c0, a0, p0)
                        if deferred_epi is not None:
                            deferred_epi()
                            deferred_epi = None
                        while fill:
                            fill.pop(0)()
                        for ci0, c0, a0, p0 in pend:
                            emit_pv(hp, ctx, nchunks, ci0, c0, a0, p0)
                        deferred_epi = make_epilogue(hp, q0, ctx)
                deferred_epi()
                attn_ctx.close()

    nc.compile()
    return nc


# ---------------------------------------------------------------------------
# host-side data prep
# ---------------------------------------------------------------------------

def _perm_cols():
    perm = np.empty(3 * D, dtype=np.int64)
    for j in range(3):
        for h in range(NH):
            for d in range(HS):
                perm[j * D + h * HS + d] = j * D + d * NH + h
    return perm


def _core_inputs(xT, W2, b2, B2, HG):
    import ml_dtypes
    bf16 = ml_dtypes.bfloat16
    cst = _phase(B2)

    def xt_slice(c):
        vs = cst[c]["vstart"]
        sl = np.zeros((D, NV2), dtype=bf16)
        lo, hi = max(0, vs), min(B * T, vs + NV2)
        sl[:, lo - vs: hi - vs] = xT[:, lo:hi].astype(bf16)
        return sl

    # weight layout: [D, hp, jj, 128]: hp halves contiguous so the hp0 half
    # can stream first
    WQK = np.empty((D, 2, 3, 128), dtype=bf16)
    BQKf = np.empty(768, dtype=np.float32)
    for jj in range(3):
        for hp in range(2):
            src = jj * D + HG * 256 + hp * 128
            WQK[:, hp, jj, :] = W2[:, src:src + 128].astype(bf16)
            fc = jj * 2 + hp
            BQKf[fc * 128:(fc + 1) * 128] = b2[src:src + 128]
    BQK = BQKf.reshape(6, 128).T.copy()  # [128, 6]: col fc, partition p

    return {
        "XTQ": xt_slice(0),
        "XTK": xt_slice(1),
        "XTV": xt_slice(2),
        "WQK": np.ascontiguousarray(WQK.reshape(D, 768)),
        "BQK": np.ascontiguousarray(BQK),
    }


# ---------------------------------------------------------------------------
# concurrent two-program dispatch (4+4 cores)
# ---------------------------------------------------------------------------

def _sharded_fn(nc, dev_lo, dev_hi):
    import jax
    from jax.sharding import Mesh, PartitionSpec
    from jax.experimental.shard_map import shard_map
    from concourse import bass2jax
    from concourse.bass2jax import _bass_exec_p, install_neuronx_cc_hook

    install_neuronx_cc_hook()
    n_cores = dev_hi - dev_lo

    in_names, out_names, out_avals, zero_shapes = [], [], [], []
    partition_name = (
        nc.partition_id_tensor.name if nc.partition_id_tensor else None
    )
    for alloc in nc.m.functions[0].allocations:
        if not isinstance(alloc, mybir.MemoryLocationSet):
            continue
        name = alloc.memorylocations[0].name
        if alloc.kind == "ExternalInput":
            if name != partition_name:
                in_names.append(name)
        elif alloc.kind == "ExternalOutput":
            np_dt = mybir.dt.np(alloc.dtype)
            out_avals.append(
                jax.core.ShapedArray(tuple(alloc.tensor_shape), np_dt)
            )
            out_names.append(name)
            zero_shapes.append((tuple(alloc.tensor_shape), np_dt))
    n_params = len(in_names)
    all_in_names = list(in_names) + list(out_names)
    if partition_name is not None:
        all_in_names.append(partition_name)

    donate = tuple(range(n_params, n_params + len(out_names)))

    def _body(*args):
        operands = list(args)
        if partition_name is not None:
            operands.append(bass2jax.partition_id_tensor())
        outs = _bass_exec_p.bind(
            *operands,
            out_avals=tuple(out_avals),
            in_names=tuple(all_in_names),
            out_names=tuple(out_names),
            lowering_input_output_aliases=(),
            sim_require_finite=True,
            sim_require_nnan=True,
            nc=nc,
        )
        return tuple(outs)

    devices = jax.devices()[dev_lo:dev_hi]
    mesh = Mesh(np.asarray(devices), ("core",))
    in_specs = (PartitionSpec("core"),) * (n_params + len(out_names))
    out_specs = (PartitionSpec("core"),) * len(out_names)
    fn = jax.jit(
        shard_map(_body, mesh=mesh, in_specs=in_specs, out_specs=out_specs,
                  check_rep=False),
        donate_argnums=donate,
        keep_unused=True,
    )
    return fn, in_names, out_names, out_avals, zero_shapes, n_cores


def _concat_inputs(in_maps, in_names):
    return [
        np.concatenate([np.asarray(m[name]) for m in in_maps], axis=0)
        for name in in_names
    ]


def kernel(x, W_qkv, b_qkv):
    x = np.asarray(x, dtype=np.float32)
    W_qkv = np.asarray(W_qkv, dtype=np.float32)
    b_qkv = np.asarray(b_qkv, dtype=np.float32)

    if "progs" not in _CACHE:
        _CACHE["progs"] = {
            B2: _build_program(B2, repeat=int(os.environ.get("KREPEAT", "1")))
            for B2 in range(2)
        }
        _CACHE["fns"] = {
            0: _sharded_fn(_CACHE["progs"][0], 0, 4),
            1: _sharded_fn(_CACHE["progs"][1], 4, 8),
        }

    perm = _perm_cols()
    W2 = W_qkv[:, perm]
    b2 = b_qkv[perm]
    xT = np.ascontiguousarray(x.reshape(B * T, D).T)

    pending = []
    for B2 in range(2):
        fn, in_names, out_names, out_avals, zero_shapes, n_cores = _CACHE["fns"][B2]
        in_maps = [_core_inputs(xT, W2, b2, B2, HG) for HG in range(4)]
        concat_in = _concat_inputs(in_maps, in_names)
        concat_zeros = [
            np.zeros((n_cores * s[0], *s[1:]), d) for (s, d) in zero_shapes
        ]
        out_arrs = fn(*concat_in, *concat_zeros)  # async dispatch
        pending.append((B2, out_names, out_avals, n_cores, out_arrs))

    out_full = np.zeros((B, T, D), dtype=np.float32)
    for B2, out_names, out_avals, n_cores, out_arrs in pending:
        per_core = np.asarray(out_arrs[0]).reshape(n_cores, T, 256)
        for HG in range(4):
            out_full[B2, :, HG * 256:(HG + 1) * 256] = per_core[HG]
    return out_full
